# revision 50
# baseline (speedup 1.0000x reference)
"""Trainium2 Bass kernel for nn_MemoryModule (sparse_attention).

Reference computation (per batch b):
  Low branch:
    mkl (9216, 64) = memory_keys_low[b] as (T*Hl*Wl, Ck)
    qkl (64, 2304) = query_key_low[b]
    A = softmax_over_n(mkl @ qkl * Ck^-0.5)          # (9216, 2304)
    memory = mvl @ A                                  # (128, 2304), mvl = (Cv, T*Hl*Wl)
  High branch:
    g_attn[t] = softmax_over_t(gk[t] @ gv[t].T * Cv^-0.5)   # (Ck, Cv) per t
    qout[t] = g_attn[t] @ qv                          # (64, 576) -> (256, 24, 24)
    qout = bilinear_upsample_2x(qout)                 # (256, 48, 48)
  out = concat([qout, memory.reshape(128, 48, 48)])   # (384, 48, 48)

Sharding: 8 cores = (b in 0..1) x (j in 0..3), where j picks 576 of the 2304
low-branch query columns (= 12 of the 48 output rows). Softmax is over the
key axis, so column blocks are independent -> no collectives.

Key optimizations vs the plain pipeline:
 - exp() writes fp8-e4m3 (with a constant logit shift of -2.5 so values fit
   the fp8 range comfortably; the shift cancels in the softmax ratio).
 - QK results land in PSUM as pair chunks (128,2,512) so one activation
   covers 1024 elements; the 64-col leftovers of 8 tiles share one bank
   (one 512-elem exp per 8 tiles). Cuts scalar-engine instruction overhead.
 - The AV matmul and the softmax denominator both consume fp8 n-tile PAIRS
   via MatmulPerfMode.DoubleRow (K=256 per instruction), halving PE columns
   for those stages. The denominator accumulates in PSUM (no DVE work).
 - The high branch is interleaved into the early loop iterations with PSUM
   pools scoped so the 8-bank budget is never exceeded.
"""

import os
import sys

for _p in ("/opt/trn_rl_repo",):
    if _p not in sys.path and os.path.isdir(_p):
        sys.path.insert(0, _p)

import numpy as np
import ml_dtypes

import concourse.bass as bass
import concourse.tile as tile
from concourse import bacc, mybir
from concourse import bass_utils

BF16 = mybir.dt.bfloat16
F32 = mybir.dt.float32
F8 = mybir.dt.float8e4

B, T, Ck, Cv = 2, 4, 64, 128
H, W, Hl, Wl = 24, 24, 48, 48
HW = H * W            # 576
NLOW = T * Hl * Wl    # 9216
MTOT = Hl * Wl        # 2304
MBLK = MTOT // 4      # 576 query columns per core
NT = NLOW // 128      # 72 n-tiles
NHALF = NT // 2       # 36
NPAIR = NT // 2       # 36 n-tile pairs for DoubleRow
HWP = 640             # 576 padded to 5*128
NC_CHUNKS = HWP // 128  # 5

SCALE_LOW = float(Ck) ** -0.5   # 0.125
BIAS_LOW = -2.5                 # constant logit shift; cancels in softmax
SCALE_HIGH = float(Cv) ** -0.5  # 0.0883883...

_PROGRAM = None
LAST_PERF = {}


def _u1d(n_in, n_out):
    """Half-pixel bilinear interpolation matrix (n_out, n_in), matches
    jax.image.resize(method='bilinear') for upsampling."""
    U = np.zeros((n_out, n_in), dtype=np.float64)
    scale = n_in / n_out
    for i in range(n_out):
        c = (i + 0.5) * scale - 0.5
        f = int(np.floor(c))
        frac = c - f
        lo = min(max(f, 0), n_in - 1)
        hi = min(max(f + 1, 0), n_in - 1)
        U[i, lo] += 1.0 - frac
        U[i, hi] += frac
    return U


def _build_upsample_full():
    """(H*W, Hl*Wl): column (ho*Wl+wo), row (h*W+w)."""
    Uh = _u1d(H, Hl)  # (48, 24)
    Uw = _u1d(W, Wl)  # (48, 24)
    Ufull = np.einsum("oh,pw->hwop", Uh, Uw).reshape(H * W, Hl * Wl)
    return Ufull.astype(np.float32)


def _build_program():
    nc = bacc.Bacc("TRN2", target_bir_lowering=False, debug=False)

    # qhead = qkl2 (576 cols) ++ mk pairs 0,1 (256 cols): one DMA gates QK(0)
    d_qhead = nc.dram_tensor("qhead", (128, MBLK + 256), BF16, kind="ExternalInput")
    d_mk = nc.dram_tensor("mk", (128, NHALF, 128), BF16, kind="ExternalInput")
    d_mv8 = nc.dram_tensor("mv8", (128, NPAIR, 2, 128), F8, kind="ExternalInput")
    d_gkT = nc.dram_tensor("gkT", (128, T, NC_CHUNKS, Ck), BF16, kind="ExternalInput")
    d_gvT = nc.dram_tensor("gvT", (128, T, NC_CHUNKS, Cv), BF16, kind="ExternalInput")
    d_qvT = nc.dram_tensor("qvT", (128, NC_CHUNKS, Cv), BF16, kind="ExternalInput")
    d_uj = nc.dram_tensor("uj", (128, NC_CHUNKS, MBLK), BF16, kind="ExternalInput")
    d_out = nc.dram_tensor("out", (T * Ck + Cv, MBLK), F32, kind="ExternalOutput")

    EXP = mybir.ActivationFunctionType.Exp
    DR = mybir.MatmulPerfMode.DoubleRow

    with tile.TileContext(nc) as tc:
        from contextlib import ExitStack

        with ExitStack() as ctx:
            cp = ctx.enter_context(tc.tile_pool(name="const", bufs=1))
            wp = ctx.enter_context(tc.tile_pool(name="work", bufs=1))

            qhead_t = cp.tile([128, MBLK + 256], BF16)
            qkl2_t = qhead_t[:, 0:MBLK]
            mk_t = cp.tile([128, NHALF, 128], BF16)
            mv8_t = cp.tile([128, NPAIR, 2, 128], F8)
            gkT_t = cp.tile([128, T, NC_CHUNKS, Ck], BF16)
            gvT_t = cp.tile([128, T, NC_CHUNKS, Cv], BF16)
            qvT_t = cp.tile([128, NC_CHUNKS, Cv], BF16)
            uj_t = cp.tile([128, NC_CHUNKS, MBLK], BF16)

            # DMA order: gate the first QK as early as possible; everything
            # else lands before its loop trigger point.
            nc.sync.dma_start(qhead_t[:], d_qhead.ap()[:, :])
            nc.sync.dma_start(mk_t[:, 2:12, :], d_mk.ap()[:, 2:12, :])
            nc.sync.dma_start(gkT_t[:], d_gkT.ap()[:, :, :, :])
            nc.sync.dma_start(gvT_t[:], d_gvT.ap()[:, :, :, :])
            nc.sync.dma_start(qvT_t[:], d_qvT.ap()[:, :, :])
            nc.sync.dma_start(uj_t[:], d_uj.ap()[:, :, :])
            nc.sync.dma_start(mk_t[:, 12:24, :], d_mk.ap()[:, 12:24, :])
            nc.sync.dma_start(mv8_t[:, 0:9, :, :], d_mv8.ap()[:, 0:9, :, :])
            nc.sync.dma_start(mk_t[:, 24:36, :], d_mk.ap()[:, 24:36, :])
            nc.sync.dma_start(mv8_t[:, 9:36, :, :], d_mv8.ap()[:, 9:36, :, :])

            ones8_t = cp.tile([128, 2, 128], F8)
            nc.gpsimd.memset(ones8_t[:], 1.0)
            bias_t = cp.tile([128, 1], F32)
            nc.gpsimd.memset(bias_t[:], BIAS_LOW)

            # ---------- high-branch emitters (interleaved into the loop) ----
            ea = []
            wts = []
            qvup_bf = wp.tile([128, MBLK], BF16)
            qops_cm = [None, None]  # (contextmanager, pool)

            def emit_ga():
                with tc.tile_pool(name="hps", bufs=2, space="PSUM") as hps:
                    for t in range(T):
                        ga = hps.tile([128, Ck], F32, name=f"ga{t}", tag="ga")
                        for c in range(NC_CHUNKS):
                            nc.tensor.matmul(
                                ga[:, :],
                                gvT_t[:, t, c, :],
                                gkT_t[:, t, c, :],
                                start=(c == 0),
                                stop=(c == NC_CHUNKS - 1),
                            )
                        e = wp.tile([128, Ck], F32, name=f"ea{t}", tag=f"ea{t}")
                        nc.scalar.activation(e[:], ga[:], EXP, scale=SCALE_HIGH)
                        ea.append(e)

            def emit_high_softmax():
                s01 = wp.tile([128, Ck], F32)
                nc.vector.tensor_add(s01[:], ea[0][:], ea[1][:])
                s23 = wp.tile([128, Ck], F32)
                nc.vector.tensor_add(s23[:], ea[2][:], ea[3][:])
                ssum = wp.tile([128, Ck], F32)
                nc.vector.tensor_add(ssum[:], s01[:], s23[:])
                rs = wp.tile([128, Ck], F32)
                nc.vector.reciprocal(rs[:], ssum[:])
                for t in range(T):
                    wt = wp.tile([128, Ck], BF16, name=f"wt{t}", tag=f"wt{t}")
                    nc.vector.tensor_mul(wt[:], ea[t][:], rs[:])
                    wts.append(wt)

            def emit_qvup():
                with tc.tile_pool(name="qvups", bufs=1, space="PSUM") as qvups:
                    qvup = qvups.tile([128, MBLK], F32)
                    for c in range(NC_CHUNKS):
                        st, sp = (c == 0), (c == NC_CHUNKS - 1)
                        nc.tensor.matmul(
                            qvup[:, 0:512], qvT_t[:, c, :], uj_t[:, c, 0:512],
                            start=st, stop=sp,
                        )
                        nc.tensor.matmul(
                            qvup[:, 512:MBLK], qvT_t[:, c, :], uj_t[:, c, 512:MBLK],
                            start=st, stop=sp,
                        )
                    nc.vector.tensor_copy(qvup_bf[:], qvup[:])

            def emit_qo_t(t):
                # one qo per loop iteration; bufs=1 is stall-free because a
                # full iteration of qk work sits between consecutive qo's.
                if t == 0:
                    cm = tc.tile_pool(name="qops", bufs=1, space="PSUM")
                    qops_cm[0] = cm
                    qops_cm[1] = cm.__enter__()
                qops = qops_cm[1]
                qo = qops.tile([Ck, MBLK], F32, name=f"qo{t}", tag="qo")
                nc.tensor.matmul(
                    qo[:, 0:512], wts[t][:, :], qvup_bf[:, 0:512],
                    start=True, stop=True,
                )
                nc.tensor.matmul(
                    qo[:, 512:MBLK], wts[t][:, :], qvup_bf[:, 512:MBLK],
                    start=True, stop=True,
                )
                qo_sb = wp.tile([Ck, MBLK], F32, name=f"qosb{t}", tag="qosb")
                nc.vector.tensor_copy(qo_sb[:], qo[:])
                nc.sync.dma_start(
                    d_out.ap()[t * Ck:(t + 1) * Ck, :], qo_sb[:])
                if t == T - 1:
                    qops_cm[0].__exit__(None, None, None)

            # ================= low branch main loop =================
            # mains: pair-chunk QK psum (128, 2, 512) -> one 1024-elem exp.
            # runts: the 64-col leftovers of 8 consecutive tiles share one
            #        bank -> one 512-elem exp per 8 tiles. Runt matmuls trail
            #        the mains by 4 tiles so the single buffer never stalls.
            # e_all: full fp8 attention matrix in SBUF; av/dn consume pairs
            #        with a 12-tile lag via DoubleRow (K=256) matmuls.
            mains = ctx.enter_context(tc.tile_pool(name="mains", bufs=2, space="PSUM"))
            runts = ctx.enter_context(tc.tile_pool(name="runts", bufs=1, space="PSUM"))
            e_all = wp.tile([128, NT, MBLK], F8)

            mn = {}
            rn = [None]

            def lhsT_of(q):
                base = 0 if q < NHALF else 64
                pair = q % NHALF
                if pair < 2:
                    return base, qhead_t[base:base + 64,
                                         MBLK + pair * 128:MBLK + (pair + 1) * 128]
                return base, mk_t[base:base + 64, pair, :]

            def emit_qk_main(q):
                p = q // 2
                if q % 2 == 0:
                    mn[p] = mains.tile([128, 2, 512], F32, name=f"mn{p}", tag="mn")
                base, lhsT = lhsT_of(q)
                nc.tensor.matmul(
                    mn[p][:, q % 2, :], lhsT,
                    qkl2_t[base:base + 64, 0:512],
                    start=True, stop=True,
                )

            def emit_qk_runt(q):
                g = q // 8
                if q % 8 == 0:
                    rn[0] = runts.tile([128, 8, 64], F32, name=f"rn{g}", tag="rn")
                base, lhsT = lhsT_of(q)
                nc.tensor.matmul(
                    rn[0][:, q % 8, :], lhsT,
                    qkl2_t[base:base + 64, 512:MBLK],
                    start=True, stop=True,
                )

            def emit_exp_main(p):
                nc.scalar.activation(
                    e_all[:, 2 * p:2 * p + 2, 0:512], mn[p][:, :, :], EXP,
                    scale=SCALE_LOW, bias=bias_t[:, 0:1])
                del mn[p]

            def emit_exp_runt(g):
                nc.scalar.activation(
                    e_all[:, 8 * g:8 * g + 8, 512:MBLK], rn[0][:, :, :], EXP,
                    scale=SCALE_LOW, bias=bias_t[:, 0:1])

            av = None
            dn = None
            a64 = None

            def consume_pair(p):
                st, sp = (p == 0), (p == NPAIR - 1)
                mvk = mv8_t[:, p, :, :]
                nc.tensor.matmul(av[:, :], mvk, e_all[:, 2 * p:2 * p + 2, 0:512],
                                 start=st, stop=sp, perf_mode=DR)
                nc.tensor.matmul(a64[:, 0, :], mvk, e_all[:, 2 * p:2 * p + 2, 512:MBLK],
                                 start=st, stop=sp, perf_mode=DR)
                nc.tensor.matmul(dn[:, :], ones8_t[:, :, :], e_all[:, 2 * p:2 * p + 2, 0:512],
                                 start=st, stop=sp, perf_mode=DR)
                nc.tensor.matmul(a64[:, 1, :], ones8_t[:, :, :], e_all[:, 2 * p:2 * p + 2, 512:MBLK],
                                 start=st, stop=sp, perf_mode=DR)

            nxt_pair = [0]

            def pair_ready_iter(p):
                # the runt exp covering tile 2p+1 is emitted at iteration
                # 8g+11 (trail-4 runts); consumable strictly after.
                return 8 * ((2 * p + 1) // 8) + 11 + 1

            def consume_ready(q, budget=2):
                while (nxt_pair[0] < NPAIR and budget > 0
                       and q >= pair_ready_iter(nxt_pair[0])):
                    consume_pair(nxt_pair[0])
                    nxt_pair[0] += 1
                    budget -= 1

            for q in range(NT):
                emit_qk_main(q)
                if q >= 4:
                    emit_qk_runt(q - 4)
                if q % 2 == 1:
                    emit_exp_main(q // 2)
                if q >= 11 and (q - 11) % 8 == 0:
                    emit_exp_runt((q - 11) // 8)

                if q == 10:
                    emit_ga()
                elif q == 12:
                    emit_high_softmax()
                elif q == 14:
                    emit_qvup()
                elif 17 <= q <= 20:
                    emit_qo_t(q - 17)
                elif q == 21:
                    avps = ctx.enter_context(
                        tc.tile_pool(name="avps", bufs=1, space="PSUM"))
                    dnps = ctx.enter_context(
                        tc.tile_pool(name="dnps", bufs=1, space="PSUM"))
                    a64ps = ctx.enter_context(
                        tc.tile_pool(name="a64ps", bufs=1, space="PSUM"))
                    av = avps.tile([128, 512], F32)
                    dn = dnps.tile([128, 512], F32)
                    a64 = a64ps.tile([128, 2, 64], F32)
                if q >= 23 and q % 2 == 1:
                    consume_ready(q)

            # drain, denominators first: the 512-col reciprocal only needs
            # the dn matmuls (no runt-exp dependency), so it overlaps the
            # AV/a64 drain instead of serializing after it.
            rcp_sb = wp.tile([128, MBLK], F32)
            rcp_scr = wp.tile([128, MBLK], F32)
            mem_sb = wp.tile([128, MBLK], F32)
            r0 = T * Ck
            p0 = nxt_pair[0]
            for p in range(p0, NPAIR):
                st, sp = (p == 0), (p == NPAIR - 1)
                nc.tensor.matmul(dn[:, :], ones8_t[:, :, :],
                                 e_all[:, 2 * p:2 * p + 2, 0:512],
                                 start=st, stop=sp, perf_mode=DR)
            nc.vector.reciprocal_approx_accurate(
                rcp_sb[:, 0:512], dn[:, :], rcp_scr[:, 0:512])
            for p in range(p0, NPAIR):
                st, sp = (p == 0), (p == NPAIR - 1)
                nc.tensor.matmul(av[:, :], mv8_t[:, p, :, :],
                                 e_all[:, 2 * p:2 * p + 2, 0:512],
                                 start=st, stop=sp, perf_mode=DR)
            for q in range(NT - 4, NT):
                emit_qk_runt(q)
            emit_exp_runt(NT // 8 - 1)
            for p in range(p0, NPAIR):
                st, sp = (p == 0), (p == NPAIR - 1)
                nc.tensor.matmul(a64[:, 0, :], mv8_t[:, p, :, :],
                                 e_all[:, 2 * p:2 * p + 2, 512:MBLK],
                                 start=st, stop=sp, perf_mode=DR)
                nc.tensor.matmul(a64[:, 1, :], ones8_t[:, :, :],
                                 e_all[:, 2 * p:2 * p + 2, 512:MBLK],
                                 start=st, stop=sp, perf_mode=DR)
            nc.vector.reciprocal_approx_accurate(
                rcp_sb[:, 512:MBLK], a64[:, 1, :], rcp_scr[:, 512:MBLK])
            nc.vector.tensor_mul(
                mem_sb[:, 0:512], av[:, :], rcp_sb[:, 0:512])
            nc.sync.dma_start(
                d_out.ap()[r0:r0 + Cv, 0:512], mem_sb[:, 0:512])
            nc.vector.tensor_mul(
                mem_sb[:, 512:MBLK], a64[:, 0, :], rcp_sb[:, 512:MBLK])
            nc.sync.dma_start(
                d_out.ap()[r0:r0 + Cv, 512:MBLK], mem_sb[:, 512:MBLK])

    nc.compile()
    return nc


def _get_program():
    global _PROGRAM
    if _PROGRAM is None:
        _PROGRAM = _build_program()
    return _PROGRAM


def _prep_core_inputs(memory_keys, memory_values, query_value,
                      memory_keys_low, memory_values_low, query_key_low,
                      Ufull, b, j):
    bf = ml_dtypes.bfloat16
    f8 = ml_dtypes.float8_e4m3

    # ---- low branch
    mk_cn = np.ascontiguousarray(
        memory_keys_low[b].transpose(1, 0, 2, 3).reshape(Ck, NLOW)
    )
    mk2 = np.concatenate([mk_cn[:, : NLOW // 2], mk_cn[:, NLOW // 2:]], axis=0)
    mk2 = np.ascontiguousarray(mk2.reshape(128, NHALF, 128)).astype(bf)

    mv_cn = memory_values_low[b].transpose(1, 0, 2, 3).reshape(Cv, NLOW)
    # (p, pair, slot, cv): slot s of pair p is n-tile 2p+s
    mv8 = np.ascontiguousarray(
        mv_cn.reshape(Cv, NPAIR, 2, 128).transpose(3, 1, 2, 0)
    ).astype(f8)

    qkl = query_key_low[b].reshape(Ck, MTOT)[:, j * MBLK:(j + 1) * MBLK]
    qkl2 = np.ascontiguousarray(np.concatenate([qkl, qkl], axis=0)).astype(bf)
    qhead = np.ascontiguousarray(np.concatenate(
        [qkl2, mk2[:, 0:2, :].reshape(128, 256)], axis=1))

    # ---- high branch (zero-padded hw -> 640 = 5*128 chunks)
    gk = memory_keys[b].reshape(T, Ck, HW)
    gkp = np.zeros((T, Ck, HWP), np.float32)
    gkp[:, :, :HW] = gk
    gkT = np.ascontiguousarray(
        gkp.reshape(T, Ck, NC_CHUNKS, 128).transpose(3, 0, 2, 1)
    ).astype(bf)  # (p, t, c, k)

    gv = memory_values[b].reshape(T, Cv, HW)
    gvp = np.zeros((T, Cv, HWP), np.float32)
    gvp[:, :, :HW] = gv
    gvT = np.ascontiguousarray(
        gvp.reshape(T, Cv, NC_CHUNKS, 128).transpose(3, 0, 2, 1)
    ).astype(bf)  # (p, t, c, v)

    qv = query_value[b].reshape(Cv, HW)
    qvp = np.zeros((Cv, HWP), np.float32)
    qvp[:, :HW] = qv
    qvT = np.ascontiguousarray(
        qvp.reshape(Cv, NC_CHUNKS, 128).transpose(2, 1, 0)
    ).astype(bf)  # (p, c, v)

    ujf = np.zeros((HWP, MBLK), np.float32)
    ujf[:HW, :] = Ufull[:, j * MBLK:(j + 1) * MBLK]
    uj = np.ascontiguousarray(
        ujf.reshape(NC_CHUNKS, 128, MBLK).transpose(1, 0, 2)
    ).astype(bf)  # (p, c, o)

    return {
        "qhead": qhead, "mk": mk2, "mv8": mv8,
        "gkT": gkT, "gvT": gvT, "qvT": qvT, "uj": uj,
    }


def kernel(memory_keys, memory_values, query_value,
           memory_keys_low, memory_values_low, query_key_low):
    memory_keys = np.asarray(memory_keys, dtype=np.float32)
    memory_values = np.asarray(memory_values, dtype=np.float32)
    query_value = np.asarray(query_value, dtype=np.float32)
    memory_keys_low = np.asarray(memory_keys_low, dtype=np.float32)
    memory_values_low = np.asarray(memory_values_low, dtype=np.float32)
    query_key_low = np.asarray(query_key_low, dtype=np.float32)

    Ufull = _build_upsample_full()
    nc = _get_program()

    in_maps = []
    for core in range(8):
        b, j = core // 4, core % 4
        in_maps.append(_prep_core_inputs(
            memory_keys, memory_values, query_value,
            memory_keys_low, memory_values_low, query_key_low, Ufull, b, j))

    trace = os.environ.get("KERNEL_TRACE", "0") == "1"
    kwargs = {}
    if trace and os.environ.get("KERNEL_TRACE_DIR"):
        os.makedirs(os.environ["KERNEL_TRACE_DIR"], exist_ok=True)
        kwargs["tmpdir"] = os.environ["KERNEL_TRACE_DIR"]
    res = bass_utils.run_bass_kernel_spmd(
        nc, in_maps, core_ids=list(range(8)), trace=trace, **kwargs
    )
    LAST_PERF.clear()
    LAST_PERF.update(
        exec_time_ns=res.exec_time_ns,
        mean_exec_time_ns=getattr(res, "mean_exec_time_ns", None),
        max_exec_time_core_id=getattr(res, "max_exec_time_core_id", None),
        per_core_scope_times=getattr(res, "per_core_scope_times", None),
        trace=getattr(res, "instructions_and_trace", None),
    )

    out = np.empty((B, T * Ck + Cv, Hl, Wl), np.float32)
    for core in range(8):
        b, j = core // 4, core % 4
        blk = res.results[core]["out"]  # (384, 576)
        out[b, :, 12 * j:12 * (j + 1), :] = blk.reshape(T * Ck + Cv, 12, Wl)
    return out
